# revision 13
# baseline (speedup 1.0000x reference)
"""Dilated-attention (SEG=512, DIL=2) fused kernel for TRN2, 8 NeuronCores.

Strategy: data-parallel over the 32 (batch, segment) attention blocks; each of
the 8 cores owns 4 blocks = 1024 active (even-position) tokens.  Odd token
positions contribute exactly zero to the output (the dilated scatter leaves
them zero before the final projection), so they are never computed.

Math folds (all exact):
  - LayerNorm mean: W is column-centered on the host, so x @ W'.T is already
    mean-free.  Only sum-of-squares is computed on device.
  - gamma on q,k: applied as a per-partition scale during PSUM eviction.
  - rstd of q,k: applied per-row/column of the score matrix before softmax.
  - rstd of v: applied per-partition while evicting the transposed attention
    weights (PT rows = key tokens).
  - gamma,beta on v: fused into the attn-output PSUM eviction (beta is exact
    because softmax rows sum to 1).
  - beta on q,k (zero in practice): rank-1 score corrections via two small
    matmuls, enabled only when beta != 0.

All big matmuls run as float32r (full-rate fp32 on the PE array for moving
dim >= 256, ~11-bit mantissa).  Every producer of fp32r-consumed data marks
its output F32R (walrus verifier requirement); DMA'd fp32 data is accepted
as-is with the PE rounding on the fly.
"""

import numpy as np

import concourse.bass as bass
import concourse.mybir as mybir
import concourse.tile as tile
from concourse import bacc
from concourse.bass_utils import run_bass_kernel_spmd

AFT = mybir.ActivationFunctionType
ALU = mybir.AluOpType
AX = mybir.AxisListType
F32 = mybir.dt.float32
F32R = mybir.dt.float32r

B, N, D = 4, 4096, 1024
SEG, DIL = 512, 2
NSEG = N // SEG          # 8 segments per batch row
S = SEG // DIL           # 256 active tokens per segment
NCORES = 8
PAIRS = B * NSEG         # 32 (b, g) attention blocks
PPC = PAIRS // NCORES    # 4 blocks per core
TOK = PPC * S            # 1024 tokens per core
NE = D // 128            # 8 feature tiles
ND = D // 128            # 8 contraction tiles
NTT = TOK // 128         # 8 token tiles
LN_EPS = 1e-5
SCALE = 1.0 / 32.0       # 1/sqrt(D)

_CACHE: dict = {}


def _build_module(beta_nonzero: bool, ln_trivial: bool, reps: int = 1,
                  loop_n: int = 0):
    nc = bacc.Bacc(
        "TRN2",
        target_bir_lowering=False,
        debug=False,
        enable_asserts=False,
        num_devices=NCORES,
    )

    def din(name, shape):
        return nc.dram_tensor(name, shape, F32, kind="ExternalInput").ap()

    aps = dict(
        xt_d=din("xt", [128, ND, TOK]),          # x.T tiles: [p, dt, tok]
        wq_d=din("wq", [NE, 128, ND, 128]),      # Wq'.T slices per e-tile
        wk_d=din("wk", [NE, 128, ND, 128]),
        wv_d=din("wv", [4, 128, ND, 256]),       # Wv'.T slices per e-quarter
        wo_d=din("wo", [128, NE, D]),            # Wo.T tiles: [p, et, f]
        bm_d=din("bmask", [128, 2, S]),          # multiplicative causal masks
        id_d=din("ident", [128, 128]),
        on_d=din("ones", [128, 1]),
        or_d=din("onerow", [8, 128]),
        gc_d=din("gcol", [128, NE]),
        bc_d=din("bcol", [128, NE]),
        ot_d=nc.dram_tensor("ot", [128, NE, PPC, S], F32,
                            kind="ExternalOutput").ap(),
    )

    with tile.TileContext(nc) as tc:
        if loop_n > 1:
            with tc.For_i(0, loop_n, 1,
                          hint_engines=(mybir.EngineType.PE,
                                        mybir.EngineType.Activation,
                                        mybir.EngineType.DVE,
                                        mybir.EngineType.SP)):
                _body(tc, beta_nonzero, ln_trivial, 0, **aps)
        else:
            for rep in range(reps):
                _body(tc, beta_nonzero, ln_trivial, rep, **aps)
    nc.compile()
    return nc


def _body(tc, beta_nonzero, ln_trivial, rep, xt_d, wq_d, wk_d, wv_d, wo_d,
          bm_d, id_d, on_d, or_d, gc_d, bc_d, ot_d):
    from contextlib import ExitStack

    nc = tc.nc
    R = f"r{rep}_"
    with ExitStack() as ctx:
        ec = ctx.enter_context

        const_p = ec(tc.tile_pool(name=R + "const", bufs=1))
        big_p = ec(tc.tile_pool(name=R + "big", bufs=1))
        qkv_p = ec(tc.tile_pool(name=R + "qkv", bufs=1))
        ws_p = ec(tc.tile_pool(name=R + "ws", bufs=3))
        scr_p = ec(tc.tile_pool(name=R + "scr", bufs=2))
        srow_p = ec(tc.tile_pool(name=R + "srow", bufs=2))
        bcast_p = ec(tc.tile_pool(name=R + "bcast", bufs=1))
        cols_p = ec(tc.tile_pool(name=R + "cols", bufs=1))
        sv4_p = ec(tc.tile_pool(name=R + "sv4", bufs=NTT))
        scol_p = ec(tc.tile_pool(name=R + "scol", bufs=8))
        att_p = ec(tc.tile_pool(name=R + "att", bufs=3))
        pt_p = ec(tc.tile_pool(name=R + "pts", bufs=3))
        yt_p = ec(tc.tile_pool(name=R + "yts", bufs=NE))
        ot_p = ec(tc.tile_pool(name=R + "ots", bufs=2))
        psA = ec(tc.tile_pool(name=R + "psA", bufs=2, space="PSUM"))
        psS = ec(tc.tile_pool(name=R + "psS", bufs=2, space="PSUM"))
        psG = ec(tc.tile_pool(name=R + "psG", bufs=2, space="PSUM"))
        psY = ec(tc.tile_pool(name=R + "psY", bufs=2, space="PSUM"))

        # ---- input first (PE start gates on it), then constants -----------
        xt_s = big_p.tile([128, ND, TOK], F32, tag="big")
        for h in range(2):
            nc.sync.dma_start(xt_s[:, :, h * 512:(h + 1) * 512].bitcast(F32R),
                              xt_d[:, :, h * 512:(h + 1) * 512].bitcast(F32R))
        gc_s = const_p.tile([128, NE], F32, tag="gc")
        nc.sync.dma_start(gc_s[:], gc_d[:])
        on_s = const_p.tile([128, 1], F32, tag="on")
        nc.sync.dma_start(on_s[:].bitcast(F32R), on_d[:].bitcast(F32R))
        bm_s = const_p.tile([128, 2, S], F32, tag="bm")
        nc.sync.dma_start(bm_s[:], bm_d[:])
        id_s = const_p.tile([128, 128], F32, tag="id")
        nc.sync.dma_start(id_s[:], id_d[:])
        or_s = const_p.tile([8, 128], F32, tag="or")
        nc.sync.dma_start(or_s[:].bitcast(F32R), or_d[:].bitcast(F32R))
        bc_s = const_p.tile([128, NE], F32, tag="bc")
        nc.sync.dma_start(bc_s[:].bitcast(F32R), bc_d[:].bitcast(F32R))
        eps_c = const_p.tile([128, 1], F32, tag="eps")
        nc.gpsimd.memset(eps_c[:], LN_EPS)

        qc_s = qkv_p.tile([128, NE, TOK], F32, tag="qc")
        kc_s = qkv_p.tile([128, NE, TOK], F32, tag="kc")
        vc_s = qkv_p.tile([128, NTT, D], F32, tag="vc")

        # ---- q/k projections + per-token sum-of-squares --------------------
        # pp[e, tok] = sum_d W'[e, d] x[tok, d]  (gamma applied on eviction;
        # squares read the unscaled PSUM).
        ssq_rows = {}
        for w_d, dst, key in ((wq_d, qc_s, "q"), (wk_d, kc_s, "k")):
            ssq = [psS.tile([1, 512], F32, tag="stat", name=f"{R}ssq_{key}{h}")
                   for h in range(2)]
            for et in range(NE):
                ws = ws_p.tile([128, ND, 128], F32, tag="wqk")
                nc.sync.dma_start(ws[:].bitcast(F32R), w_d[et].bitcast(F32R))
                for h in range(2):
                    pp = psA.tile([128, 512], F32, tag="proj")
                    for dt in range(ND):
                        nc.tensor.matmul(
                            pp[:],
                            ws[:, dt, :].bitcast(F32R),
                            xt_s[:, dt, h * 512:(h + 1) * 512].bitcast(F32R),
                            start=(dt == 0), stop=(dt == ND - 1),
                        )
                    nc.vector.tensor_scalar_mul(
                        dst[:, et, h * 512:(h + 1) * 512].bitcast(F32R), pp[:],
                        gc_s[:, et:et + 1])
                    sq = scr_p.tile([128, 512], F32, tag="sq")
                    nc.scalar.activation(sq[:].bitcast(F32R), pp[:], AFT.Square)
                    nc.tensor.matmul(
                        ssq[h][:], on_s[:].bitcast(F32R), sq[:].bitcast(F32R),
                        start=(et == 0), stop=(et == NE - 1),
                    )
            ssq_rows[key] = ssq

        # ---- q rstd: rq32 = 1/(32*sqrt(ssq/D + eps)), as [128, 8] columns --
        rq32_col = cols_p.tile([128, 2 * PPC], F32, tag="rq32")
        rq32_row = []
        for h in range(2):
            t = srow_p.tile([1, 512], F32, tag="rowa", bufs=1)
            nc.scalar.activation(t[:], ssq_rows["q"][h][:], AFT.Identity,
                                 bias=eps_c[:1, :], scale=1.0 / D)
            s = srow_p.tile([1, 512], F32, tag="rowb", bufs=1)
            # sqrt(1024 * t) = 32*sqrt(t); reciprocal gives rstd/32 directly
            nc.scalar.activation(s[:], t[:], AFT.Sqrt, bias=0.0, scale=float(D))
            r = srow_p.tile([1, 512], F32, tag="rowc")
            nc.vector.reciprocal(r[:], s[:])
            rq32_row.append(r)
        # column-ize: out[:, i] = row chunk i via K=1 fp32 matmuls
        pcol = psS.tile([128, 2 * PPC], F32, tag="stat", name=R + "pcol_q")
        for i in range(2 * PPC):
            h, j = divmod(i, PPC)
            nc.tensor.matmul(pcol[:, i:i + 1],
                             rq32_row[h][:, j * 128:(j + 1) * 128],
                             or_s[0:1, 0:1], start=True, stop=True)
        nc.scalar.activation(rq32_col[:], pcol[:], AFT.Copy)

        # ---- k rstd: replicated to all partitions as rkb [128, TOK] --------
        rkb_s = bcast_p.tile([128, TOK], F32, tag="rkb")
        for h in range(2):
            sk = srow_p.tile([1, 512], F32, tag="rowa", bufs=1)
            nc.scalar.activation(sk[:].bitcast(F32R), ssq_rows["k"][h][:],
                                 AFT.Copy)
            pr = psA.tile([128, 512], F32, tag="proj")
            nc.tensor.matmul(pr[:], or_s[0:1, :].bitcast(F32R),
                             sk[:].bitcast(F32R))
            t = scr_p.tile([128, 512], F32, tag="sq")
            nc.scalar.activation(t[:], pr[:], AFT.Identity,
                                 bias=eps_c[:], scale=1.0 / D)
            s2 = scr_p.tile([128, 512], F32, tag="sq")
            nc.scalar.activation(s2[:], t[:], AFT.Sqrt)
            nc.vector.reciprocal(rkb_s[:, h * 512:(h + 1) * 512], s2[:])

        # ---- v projection + sum-of-squares over the free dim ---------------
        sv4 = [sv4_p.tile([128, 4], F32, tag="sv4", name=f"{R}sv4_{tt}")
               for tt in range(NTT)]
        for qtr in range(4):
            wvs = ws_p.tile([128, ND, 256], F32, tag="wqk")
            nc.sync.dma_start(wvs[:].bitcast(F32R), wv_d[qtr].bitcast(F32R))
            for tt in range(NTT):
                pv = psA.tile([128, 256], F32, tag="proj")
                for dt in range(ND):
                    nc.tensor.matmul(
                        pv[:],
                        xt_s[:, dt, tt * 128:(tt + 1) * 128].bitcast(F32R),
                        wvs[:, dt, :].bitcast(F32R),
                        start=(dt == 0), stop=(dt == ND - 1),
                    )
                vslc = vc_s[:, tt, qtr * 256:(qtr + 1) * 256]
                nc.vector.tensor_copy(vslc.bitcast(F32R), pv[:])
                # sum-of-squares of the unrounded PSUM on ACT
                sqv = scr_p.tile([128, 256], F32, tag="sqv")
                nc.scalar.activation(sqv[:], pv[:], AFT.Square,
                                     accum_out=sv4[tt][:, qtr:qtr + 1])

        # ---- v rstd in column form [128, NTT] ------------------------------
        sv_col = cols_p.tile([128, NTT], F32, tag="svc")
        for tt in range(NTT):
            nc.vector.reduce_sum(sv_col[:, tt:tt + 1], sv4[tt][:], axis=AX.X)
        svt = cols_p.tile([128, NTT], F32, tag="svt")
        nc.scalar.activation(svt[:], sv_col[:], AFT.Identity,
                             bias=eps_c[:], scale=1.0 / D)
        svs = cols_p.tile([128, NTT], F32, tag="svs")
        nc.scalar.activation(svs[:], svt[:], AFT.Sqrt)
        rv_col = cols_p.tile([128, NTT], F32, tag="rvc")
        nc.vector.reciprocal(rv_col[:], svs[:])

        # ---- beta corrections for q,k scores (skipped when beta == 0) ------
        # S += u[i] + w[j] + |beta|^2 with u = rstd_q * (beta . gamma*qc),
        # w likewise for k.
        u32_col = None
        wb_s = None
        if beta_nonzero:
            # |beta|^2 / 32 as a [1,1] scalar
            bsq = cols_p.tile([128, NE], F32, tag="bsq")
            nc.scalar.activation(bsq[:], bc_s[:], AFT.Square)
            pbb = psS.tile([1, NE], F32, tag="stat")
            nc.tensor.matmul(pbb[:], on_s[:], bsq[:])
            bb8 = srow_p.tile([1, NE], F32, tag="bb8")
            nc.scalar.activation(bb8[:], pbb[:], AFT.Copy)
            bb32 = srow_p.tile([1, 1], F32, tag="bb32")
            nc.vector.tensor_reduce(bb32[:], bb8[:], axis=AX.X, op=ALU.add)
            nc.vector.tensor_scalar_mul(bb32[:], bb32[:], SCALE)

            mrow = {}
            for src, key in ((qc_s, "q"), (kc_s, "k")):
                pm = [psS.tile([1, 512], F32, tag="stat", name=f"{R}pm_{key}{h}")
                      for h in range(2)]
                for et in range(NE):
                    for h in range(2):
                        nc.tensor.matmul(
                            pm[h][:], bc_s[:, et:et + 1].bitcast(F32R),
                            src[:, et, h * 512:(h + 1) * 512].bitcast(F32R),
                            start=(et == 0), stop=(et == NE - 1),
                        )
                rows = []
                for h in range(2):
                    mr = srow_p.tile([1, 512], F32, tag="rowd")
                    nc.scalar.activation(mr[:], pm[h][:], AFT.Copy)
                    rows.append(mr)
                mrow[key] = rows

            u32_col = cols_p.tile([128, 2 * PPC], F32, tag="u32")
            urows = []
            for h in range(2):
                u = srow_p.tile([1, 512], F32, tag="rowe", bufs=1)
                nc.vector.scalar_tensor_tensor(
                    u[:], mrow["q"][h][:], 1.0, rq32_row[h][:],
                    op0=ALU.bypass, op1=ALU.mult)
                urows.append(u)
            pcu = psS.tile([128, 2 * PPC], F32, tag="stat", name=R + "pcol_u")
            for i in range(2 * PPC):
                h, j = divmod(i, PPC)
                nc.tensor.matmul(pcu[:, i:i + 1],
                                 urows[h][:, j * 128:(j + 1) * 128],
                                 or_s[0:1, 0:1], start=True, stop=True)
            nc.scalar.activation(u32_col[:], pcu[:], AFT.Copy)

            # w row = mk * rk / 32 + |b|^2/32, replicated to wb [128, TOK]
            wb_s = bcast_p.tile([128, TOK], F32, tag="wb")
            for h in range(2):
                w1 = srow_p.tile([1, 512], F32, tag="rowf", bufs=1)
                nc.vector.scalar_tensor_tensor(
                    w1[:], mrow["k"][h][:], SCALE,
                    rkb_s[0:1, h * 512:(h + 1) * 512],
                    op0=ALU.mult, op1=ALU.mult)
                w2 = srow_p.tile([1, 512], F32, tag="rowg", bufs=1)
                nc.vector.tensor_scalar(w2[:].bitcast(F32R), w1[:], bb32[:],
                                        None, op0=ALU.add)
                pw = psA.tile([128, 512], F32, tag="proj")
                nc.tensor.matmul(pw[:], or_s[0:1, :].bitcast(F32R),
                                 w2[:].bitcast(F32R))
                nc.scalar.activation(wb_s[:, h * 512:(h + 1) * 512], pw[:],
                                     AFT.Copy)

        # ---- attention + output projection, per segment --------------------
        wo_s = big_p.tile([128, NE, D], F32, tag="big")
        nc.sync.dma_start(wo_s[:].bitcast(F32R), wo_d[:].bitcast(F32R))

        for sg in range(PPC):
            c0 = sg * 2 * 128
            pts = [pt_p.tile([128, 2 * 128], F32, tag="pt", name=f"{R}pts_{sg}_{kt}")
                   for kt in range(2)]
            for qt in range(2):
                g = psA.tile([128, 256], F32, tag="proj")
                q0 = c0 + qt * 128
                for et in range(NE):
                    nc.tensor.matmul(
                        g[:],
                        qc_s[:, et, q0:q0 + 128].bitcast(F32R),
                        kc_s[:, et, c0:c0 + 256].bitcast(F32R),
                        start=(et == 0), stop=(et == NE - 1),
                    )
                # T = G * rq32[i] * rk[j]  (scores / 32)
                t1 = att_p.tile([128, 256], F32, tag="t1")
                nc.vector.scalar_tensor_tensor(
                    t1[:], g[:], rq32_col[:, 2 * sg + qt:2 * sg + qt + 1],
                    rkb_s[:, c0:c0 + 256], op0=ALU.mult, op1=ALU.mult)
                if beta_nonzero:
                    t1b = att_p.tile([128, 256], F32, tag="p3")
                    nc.vector.scalar_tensor_tensor(
                        t1b[:], t1[:],
                        u32_col[:, 2 * sg + qt:2 * sg + qt + 1],
                        wb_s[:, c0:c0 + 256], op0=ALU.add, op1=ALU.add)
                    t1 = t1b
                p = att_p.tile([128, 256], F32, tag="p")
                if ln_trivial:
                    # |scores/32| <= 32 -> exp is safe without the row-max
                    # subtraction
                    nc.scalar.activation(p[:], t1[:], AFT.Exp, bias=0.0,
                                         scale=1.0)
                else:
                    negm = scol_p.tile([128, 1], F32, tag="negm")
                    nc.vector.tensor_reduce(negm[:], t1[:], axis=AX.X,
                                            op=ALU.max, negate=True)
                    nc.scalar.activation(p[:], t1[:], AFT.Exp, bias=negm[:],
                                         scale=1.0)
                # mask + masked row-sum in one op
                l = scol_p.tile([128, 1], F32, tag="l")
                pz = att_p.tile([128, 256], F32, tag="t1")
                nc.vector.scalar_tensor_tensor(
                    pz[:], p[:], 1.0, bm_s[:, qt, :],
                    op0=ALU.bypass, op1=ALU.mult, accum_out=l[:])
                rl = scol_p.tile([128, 1], F32, tag="rl")
                nc.vector.reciprocal(rl[:], l[:])
                # P3 = P * (1/l)[i]
                p3 = att_p.tile([128, 256], F32, tag="p3")
                nc.vector.tensor_scalar_mul(p3[:], pz[:], rl[:])
                for kt in range(2):
                    ptp = psG.tile([128, 128], F32, tag="att")
                    nc.tensor.transpose(ptp[:], p3[:, kt * 128:(kt + 1) * 128],
                                        id_s[:])
                    # rstd_v applied per key token (partition dim of PT)
                    nc.scalar.activation(
                        pts[kt][:, qt * 128:(qt + 1) * 128].bitcast(F32R),
                        ptp[:], AFT.Copy, bias=0.0,
                        scale=rv_col[:, sg * 2 + kt:sg * 2 + kt + 1])

            # y.T[e, qt] = sum_kt vc[kt, e] * PT[kt, qt]; gamma/beta fused
            # into the eviction (exact: softmax rows sum to 1).
            yts = [yt_p.tile([128, 256], F32, tag="yt", name=f"{R}yts_{sg}_{et}")
                   for et in range(NE)]
            for et in range(NE):
                yp = psY.tile([128, 256], F32, tag="yo")
                for kt in range(2):
                    nc.tensor.matmul(
                        yp[:],
                        vc_s[:, sg * 2 + kt, et * 128:(et + 1) * 128].bitcast(F32R),
                        pts[kt][:].bitcast(F32R),
                        start=(kt == 0), stop=(kt == 1),
                    )
                nc.vector.tensor_scalar(yts[et][:].bitcast(F32R), yp[:],
                                        gc_s[:, et:et + 1], bc_s[:, et:et + 1],
                                        op0=ALU.mult, op1=ALU.add)

            # out.T[f, qt] = sum_e Wo[f, e] y[qt, e]
            for ft in range(NE):
                po = psY.tile([128, 256], F32, tag="yo")
                for et in range(NE):
                    nc.tensor.matmul(
                        po[:],
                        wo_s[:, et, ft * 128:(ft + 1) * 128].bitcast(F32R),
                        yts[et][:].bitcast(F32R),
                        start=(et == 0), stop=(et == NE - 1),
                    )
                ots = ot_p.tile([128, 256], F32, tag="ot")
                nc.vector.tensor_copy(ots[:], po[:])
                nc.sync.dma_start(ot_d[:, ft, sg, :], ots[:])


def _get_module(beta_nonzero: bool, ln_trivial: bool, reps: int = 1,
                loop_n: int = 0):
    key = ("mod", beta_nonzero, ln_trivial, reps, loop_n)
    if key not in _CACHE:
        _CACHE[key] = _build_module(beta_nonzero, ln_trivial, reps, loop_n)
    return _CACHE[key]


def _host_prep(x, Wq, Wk, Wv, Wo, gamma, beta):
    """Build per-core input dicts (numpy only)."""
    x = np.ascontiguousarray(np.asarray(x, dtype=np.float32))
    Wq = np.asarray(Wq, dtype=np.float32)
    Wk = np.asarray(Wk, dtype=np.float32)
    Wv = np.asarray(Wv, dtype=np.float32)
    Wo = np.asarray(Wo, dtype=np.float32)
    gamma = np.asarray(gamma, dtype=np.float32)
    beta = np.asarray(beta, dtype=np.float32)

    # dilated gather: [B, NSEG, S, D] -> [PAIRS, S, D]
    xs = x.reshape(B, NSEG, SEG, D)[:, :, ::DIL, :].reshape(PAIRS, S, D)

    def center_T(W):
        Wc = W - W.mean(axis=0, keepdims=True)       # subtract per-d col mean
        return np.ascontiguousarray(Wc.T)            # [d, e]

    WqT = center_T(Wq)
    WkT = center_T(Wk)
    WvT = center_T(Wv)
    WoT = np.ascontiguousarray(Wo.T)                 # [e, f]

    # weight slice layouts
    wq_h = np.ascontiguousarray(
        WqT.reshape(ND, 128, NE, 128).transpose(2, 1, 0, 3))   # [et,p,dt,el]
    wk_h = np.ascontiguousarray(
        WkT.reshape(ND, 128, NE, 128).transpose(2, 1, 0, 3))
    wv_h = np.ascontiguousarray(
        WvT.reshape(ND, 128, 4, 256).transpose(2, 1, 0, 3))    # [qtr,p,dt,el]
    wo_h = np.ascontiguousarray(
        WoT.reshape(NE, 128, D).transpose(1, 0, 2))            # [p, et, f]

    # multiplicative causal masks for the two query tiles of a segment
    r = np.arange(128)[:, None]
    c = np.arange(S)[None, :]
    bm = np.zeros((128, 2, S), dtype=np.float32)
    bm[:, 0, :] = (c <= r).astype(np.float32)
    bm[:, 1, :] = (c <= r + 128).astype(np.float32)

    ident = np.eye(128, dtype=np.float32)
    ones = np.ones((128, 1), dtype=np.float32)
    onerow = np.ones((8, 128), dtype=np.float32)
    gcol = np.ascontiguousarray(gamma.reshape(NE, 128).T)      # [p, et]
    bcol = np.ascontiguousarray(beta.reshape(NE, 128).T)

    shared = {
        "wq": wq_h, "wk": wk_h, "wv": wv_h, "wo": wo_h,
        "bmask": bm, "ident": ident, "ones": ones, "onerow": onerow,
        "gcol": gcol, "bcol": bcol,
    }

    in_maps = []
    for cidx in range(NCORES):
        toks = xs[cidx * PPC:(cidx + 1) * PPC].reshape(TOK, D)   # [t, d]
        xT = toks.T                                              # [d, t]
        xt_h = np.ascontiguousarray(
            xT.reshape(ND, 128, TOK).transpose(1, 0, 2))         # [p, dt, t]
        m = dict(shared)
        m["xt"] = xt_h
        in_maps.append(m)
    beta_nonzero = bool(np.any(beta != 0.0))
    ln_trivial = (not beta_nonzero) and bool(np.all(gamma == 1.0))
    return in_maps, beta_nonzero, ln_trivial


def _assemble(results):
    """[core]["ot"]: [128, NE, PPC, S] -> full [B, N, D] output."""
    out = np.zeros((PAIRS, SEG, D), dtype=np.float32)
    for cidx in range(NCORES):
        ot = results[cidx]["ot"]                     # [p, ft, sg, j]
        arr = ot.transpose(2, 3, 1, 0).reshape(PPC, S, D)   # [sg, j, f]
        out[cidx * PPC:(cidx + 1) * PPC, ::DIL, :] = arr
    return out.reshape(B, N, D)


def kernel(x, Wq, Wk, Wv, Wo, gamma, beta):
    in_maps, beta_nonzero, ln_trivial = _host_prep(x, Wq, Wk, Wv, Wo, gamma,
                                                   beta)
    nc = _get_module(beta_nonzero, ln_trivial)
    res = run_bass_kernel_spmd(nc, in_maps, core_ids=list(range(NCORES)))
    return _assemble(res.results)
